# revision 47
# baseline (speedup 1.0000x reference)
"""Trainium2 Bass kernel for nn_Attention_47493748359201.

Single-head attention: q/k/v projections -> softmax(q k^T) v -> output proj.
Full shapes: query/keys/values [4, 2048, 1024], weights [1024, 1024].

Sharding: 8 cores = (batch, query-half). Each core handles its own
1024-row query slice against the full 2048 keys of its batch; no
collectives.

Algebraic folds (exact, by matmul associativity):
  - scores = (Xq Wq)(Xk Wk)^T = Xq M Xk^T with M = Wq Wk^T precomputed
    on the host.  The kernel projects keys once through M^T (same cost
    as the old K projection) and uses raw Xq^T as the score stationary.
    The Q projection disappears.
  - out = attn (Xv Wv + bv) Wd + bd = attn Xv N + b' with N = Wv Wd and
    b' = bv Wd + bd precomputed on the host.  The attend matmul uses raw
    Xv (bf16) as its stationary.  The V projection disappears.
  Old tensor-engine work per core: kproj+qproj+vproj+scores+attend+out
  = 273us ideal; new: t''proj+scores+attend+out = 191us ideal.

Precision: single-pass fp32r (11-bit mantissa) for the t'' projection
and the score matmul; bf16 for attend and output-proj (softmax weights
in [0,1]; Xv and attended tolerate 8-bit mantissa).

Bias handling (exact):
  - bk: the scores term q.bk is constant along the key axis -> drops out
    of softmax; bq.bk is a global constant -> drops too.
  - bq: enters only through colbias[sk] = bq . k[sk] = keys[sk].(Wk bq),
    computed exactly on the host (tiny matvec) and added to scores.
  - bv, bd: fold into b' = bv Wd + bd, added (f32) during the output
    PSUM eviction.

Layout: everything SBUF-resident (no DRAM staging).  One "big" pool
with eight 16KB/partition slots rotates the long-lived tensors; ae/ao
(attended^T) reuse t'' slots after the score phase.

HW-measured scheduling rules this kernel is built around (verified by
same-window A/B on the device; the cost model does NOT predict them):
  - Split every long-lived tensor into even/odd tiles along the axis its
    consumer's matmul chain cycles through: alternating the moving
    operand between two SBUF tiles streams ~2x faster.
  - Sequential psum accumulation chains only.  Interleaving two chains
    or widening chunks to 512 both measured ~25% slower.
  - Evictions must alternate engines (ACT/DVE): a single-engine eviction
    stream gates psum recycling.
  - Keep DMAs on the sync queue in fine per-do slices; the gpsimd queue
    and consolidated strided DMAs both regressed.
"""
import sys

sys.path.insert(0, "/opt/trn_rl_repo")

import numpy as np
import ml_dtypes

import concourse.bass as bass
import concourse.mybir as mybir
import concourse.tile as tile
from concourse import bacc
from concourse.masks import make_identity

P = 128
NB = 512  # matmul moving free dim (one PSUM bank of f32)
AF = mybir.ActivationFunctionType
ALU = mybir.AluOpType
dt = mybir.dt
f32 = dt.float32
f32r = dt.float32r
bf16 = dt.bfloat16

# full-problem constants
B, S, D, H, DEP = 4, 2048, 1024, 1024, 1024
NCORES = 8
SQ = B * S // NCORES  # 1024 query rows per core


def input_specs(S=S, D=D, DEP=DEP, SQ=SQ):
    """name -> (shape, mybir dtype) for the per-core DRAM inputs."""
    DT, SKT = D // P, S // P
    return {
        "xq": ([P, DT, SQ], f32r),       # query^T  (d-major tiles)
        "xk": ([P, DT, S], f32r),        # keys^T   (d-major tiles)
        # values, regrouped per e-block and split by sk-tile parity so the
        # attend phase streams one contiguous [P, SKT/2*P] slice per eo:
        # xv0[p, eo, skoh*P + j] = values[(2*skoh)*P + p, eo*P + j]
        "xv0": ([P, DT, SKT // 2 * P], bf16),
        "xv1": ([P, DT, SKT // 2 * P], bf16),
        "wm": ([P, DT, D], f32r),        # M^T = Wk Wq^T (d_in-major)
        "wn": ([P, DT, DEP], bf16),      # N = Wv Wd     (d-major)
        "bp": ([P, DEP], f32),           # b' = bv Wd + bd (broadcast)
        "colbias": ([P, S], bf16),
    }


def emit_attention(ctx, tc, io, S=S, D=D, DEP=DEP, SQ=SQ, upto=3):
    """Emit the per-core attention program. io: dict name -> bass.AP
    (input_specs() names plus "out" [SQ, DEP] f32).
    upto: emit phases 0..upto only (for phase timing)."""
    nc = tc.nc
    DT, SKT, SQT = D // P, S // P, SQ // P
    SKC = S // NB          # score/key column chunks (4)
    S2 = S // 2            # t'' cols per half tile
    DC = DEP // NB         # output dep chunks
    NBQ = 512              # t''-projection column chunk
    SQC = min(NB, SQ)      # attend rhs chunk
    NSQC = SQ // SQC
    D2W = D // 2           # M-half width (512)
    NWH = D // D2W
    DT_W = D2W // P        # d tiles per M half
    XSB = 3                # xs-tag stream depth (chunks are 2MB at NBQ=512)

    # ---------------- resident SBUF (whole kernel) ----------------
    res = ctx.enter_context(tc.tile_pool(name="res", bufs=1))
    ident = res.tile([P, P], bf16)
    colbias = res.tile([P, S], bf16)
    nc.sync.dma_start(colbias[:], io["colbias"])
    bp_t = res.tile([P, DEP], f32)
    nc.sync.dma_start(bp_t[:], io["bp"])

    # big rotating slots (bufs=6 x 16KB/part).  Long-lived tensors are
    # split into even/odd tiles along the axis their consumer's matmul
    # chain cycles through, so consecutive matmuls alternate SBUF tiles.
    big = ctx.enter_context(tc.tile_pool(name="big", bufs=6))
    # t''[parity of do][colhalf]: scores chunk c cycles do, alt ke/ko
    ke0 = big.tile([P, DT // 2, S2], f32r, tag="big")
    ko0 = big.tile([P, DT // 2, S2], f32r, tag="big")
    ke1 = big.tile([P, DT // 2, S2], f32r, tag="big")
    ko1 = big.tile([P, DT // 2, S2], f32r, tag="big")
    # attnT[parity of sko]: attend chain cycles sko
    aT0 = big.tile([P, SKT // 2, SQ], bf16, tag="big")
    aT1 = big.tile([P, SKT // 2, SQ], bf16, tag="big")

    ps = ctx.enter_context(tc.tile_pool(name="ps", bufs=1, space="PSUM"))
    # stream pool: persistent tags -> DMA prefetch crosses phase boundaries
    strm = ctx.enter_context(tc.tile_pool(name="strm", bufs=1))

    ident_f = strm.tile([P, P], f32, name="ident_f", tag="vo", bufs=2)
    make_identity(nc, ident_f[:])
    nc.vector.tensor_copy(ident[:], ident_f[:])

    def load_whalf(nm, w_ap, half, dtp=f32r):
        """One M half as an even/odd-do pair of tiles."""
        te = strm.tile([P, DT // 2, D2W], dtp, name=nm + "e", tag="w",
                       bufs=4)
        to = strm.tile([P, DT // 2, D2W], dtp, name=nm + "o", tag="w",
                       bufs=4)
        hs = slice(half * D2W, (half + 1) * D2W)
        for do in range(DT):
            t = te if do % 2 == 0 else to
            nc.sync.dma_start(t[:, do // 2, :], w_ap[:, do, hs])
        return te, to

    def load_x_chunk(x_ap, c, ncols=NBQ):
        cs = slice(c * ncols, (c + 1) * ncols)
        xe = strm.tile([P, DT // 2, ncols], f32r, name="xe", tag="xs",
                       bufs=XSB)
        xo = strm.tile([P, DT // 2, ncols], f32r, name="xo", tag="xs",
                       bufs=XSB)
        for do in range(DT):
            t = xe if do % 2 == 0 else xo
            nc.sync.dma_start(t[:, do // 2, :], x_ap[:, do, cs])
        return xe, xo

    def proj(w_halves, x_ap, dst_of, ncols, first_x=None):
        """dst[i, col] = W^T @ x, single-pass f32r.
        dst_of(c) -> (tile, col_slice) eviction target for chunk c;
        dst tile is an (even, odd) i-parity pair."""
        for c in range(ncols // NBQ):
            xe, xo = first_x if (c == 0 and first_x is not None) \
                else load_x_chunk(x_ap, c)
            for ho in range(DT):
                pt = ps.tile([P, NBQ], f32, tag="mm", name="pt", bufs=2)
                we, wo = w_halves[ho // DT_W]
                hs = slice((ho % DT_W) * P, (ho % DT_W + 1) * P)
                for do in range(DT):
                    wt = we if do % 2 == 0 else wo
                    xt = xe if do % 2 == 0 else xo
                    nc.tensor.matmul(pt[:], wt[:, do // 2, hs],
                                     xt[:, do // 2, :],
                                     start=(do == 0), stop=(do == DT - 1))
                dst_pair, cs = dst_of(c)
                dst = dst_pair[ho % 2][:, ho // 2, cs]
                # alternate eviction engine: halves the serial drain at
                # phase handoffs (consumers wait on the dst tile writers)
                if ho % 2 == 0:
                    nc.scalar.activation(dst, pt[:], AF.Copy)
                else:
                    nc.vector.tensor_copy(dst, pt[:])

    # ------------- phase 0: t'' = M^T-projection of keys -------------
    # first M half, then the first xk chunk, then the second half, so
    # the first psum group starts after ~3MB of DMA instead of ~5MB
    wm_h = [load_whalf("wm0", io["wm"], 0)]
    kfirst = load_x_chunk(io["xk"], 0)
    wm_h += [load_whalf(f"wm{h}", io["wm"], h) for h in range(1, NWH)]

    def k_dst(c):
        pair = (ke0, ko0) if c < S2 // NBQ else (ke1, ko1)
        c0 = c % (S2 // NBQ)
        return pair, slice(c0 * NBQ, (c0 + 1) * NBQ)

    proj(wm_h, io["xk"], k_dst, S, first_x=kfirst)
    if upto == 0:
        return

    # ---------------- phase 1: scores + softmax + transpose ----------------
    # Software-pipelined: sqt's transposes are emitted AFTER sqt+1's score
    # chains so the in-order tensor queue never waits on sqt's softmax.
    # Engine split per chunk: PE chain -> Pool (colbias add + max) -> ACT
    # (exp).  DVE only runs the tiny normalize chain + half the rescales
    # and aT copies; without this the phase ran serial at ~123us/body
    # (PE 61 + DVE 60 + ACT 15 all on the critical path).
    EW = min(S, 2 * NB)  # columns per e tile
    with tc.tile_pool(name="soft", bufs=2) as soft:
        def score_block(sqt):
            sq0 = sqt * P
            # stationary for this query block: xq[:, do, sq0:sq0+P],
            # split by do parity
            xqe = strm.tile([P, DT // 2, P], f32r, name="xqe", tag="xqe",
                            bufs=2)
            xqo = strm.tile([P, DT // 2, P], f32r, name="xqo", tag="xqo",
                            bufs=2)
            # one strided DMA per parity: tiny per-do slices are DMA-
            # latency-bound (~2.3us each regardless of size)
            nc.sync.dma_start(xqe[:], io["xq"][:, 0::2, sq0:sq0 + P])
            nc.sync.dma_start(xqo[:], io["xq"][:, 1::2, sq0:sq0 + P])
            es_ = [strm.tile([P, EW], bf16, name="e", tag="e", bufs=4)
                   for _ in range(S // EW)]
            nm_arr = soft.tile([P, SKC], f32, name="nm_arr")
            es_arr = soft.tile([P, SKC], f32, name="es_arr")
            for c in range(SKC):
                cs = slice(c * NB, (c + 1) * NB)
                kpair = (ke0, ko0) if c < SKC // 2 else (ke1, ko1)
                kcs = slice((c % (SKC // 2)) * NB, (c % (SKC // 2) + 1) * NB)
                sch = ps.tile([P, NB], f32, tag="sc", name="sch", bufs=4)
                for do in range(DT):
                    xqt = xqe if do % 2 == 0 else xqo
                    nc.tensor.matmul(sch[:], xqt[:, do // 2, :],
                                     kpair[do % 2][:, do // 2, kcs],
                                     start=(do == 0), stop=(do == DT - 1))
                # colbias add evicts PSUM->SBUF in the same pass: the
                # max and exp then read SBUF (PSUM-side reads cost ~2x on
                # DVE/ACT) and the PSUM bank frees one op earlier
                sb = strm.tile([P, NB], f32, name="sb", tag="sb", bufs=4)
                nc.vector.tensor_tensor(sb[:], sch[:], colbias[:, cs],
                                        ALU.add)
                nc.vector.reduce_max(out=nm_arr[:, c:c + 1], in_=sb[:],
                                     axis=mybir.AxisListType.X, negate=True)
                ei = es_[(c * NB) // EW]
                ecs = slice((c * NB) % EW, (c * NB) % EW + NB)
                nc.scalar.activation(ei[:, ecs], sb[:], AF.Exp,
                                     bias=nm_arr[:, c:c + 1],
                                     accum_out=es_arr[:, c:c + 1])
            # global max and per-quarter rescale factors
            nmax = soft.tile([P, 1], f32, name="nmax")
            nc.vector.tensor_reduce(out=nmax[:], in_=nm_arr[:],
                                    op=ALU.min, axis=mybir.AxisListType.X)
            dm = soft.tile([P, SKC], f32, name="dm")
            nc.vector.tensor_scalar_sub(dm[:], nm_arr[:], nmax[:])
            fq = soft.tile([P, SKC], f32, name="fq")
            nc.scalar.activation(fq[:], dm[:], AF.Exp, scale=-1.0)
            wsum = soft.tile([P, SKC], f32, name="wsum")
            nc.vector.tensor_tensor(wsum[:], fq[:], es_arr[:], ALU.mult)
            esum = soft.tile([P, 1], f32, name="esum")
            nc.vector.reduce_sum(out=esum[:], in_=wsum[:],
                                 axis=mybir.AxisListType.X)
            recip = soft.tile([P, 1], f32, name="recip")
            nc.vector.reciprocal(recip[:], esum[:])
            r_arr = soft.tile([P, SKC], f32, name="r_arr")
            nc.vector.tensor_scalar_mul(r_arr[:], fq[:], recip[:])
            for c in range(SKC):
                ei = es_[(c * NB) // EW]
                ecs = slice((c * NB) % EW, (c * NB) % EW + NB)
                nc.vector.tensor_scalar_mul(ei[:, ecs], ei[:, ecs],
                                            r_arr[:, c:c + 1])
            return es_, sq0

        def transpose_block(es_, sq0):
            # two same-parity transposes share one [P, 2P] psum tile and
            # drain in ONE strided 256-col copy (8 copies/block, not 16)
            for par in range(2):
                aT = aT0 if par == 0 else aT1
                for k2 in range(SKT // 4):
                    ptr = ps.tile([P, 2, P], bf16, tag="tr", name="ptr",
                                  bufs=2)
                    for h in range(2):
                        sko = (2 * k2 + h) * 2 + par
                        ei = es_[(sko * P) // EW]
                        ecs = slice((sko * P) % EW, (sko * P) % EW + P)
                        nc.tensor.transpose(ptr[:, h, :], ei[:, ecs],
                                            ident[:])
                    dst = aT[:, 2 * k2:2 * k2 + 2, sq0:sq0 + P]
                    if (par + k2) % 2 == 0:
                        nc.vector.tensor_copy(dst, ptr[:])
                    else:
                        nc.scalar.activation(dst, ptr[:], AF.Copy)

        pend = None
        for sqt in range(SQT):
            blk = score_block(sqt)
            if pend is not None:
                transpose_block(*pend)
            pend = blk
        transpose_block(*pend)
    if upto == 1:
        return

    # ------------- phase 2: attend (attG^T into t'' slots) -------------
    # attG^T[parity of eo]: outproj's stationary cycles eo.  The xv
    # stationary streams from DRAM one eo-block at a time (re-loaded per
    # sqc pass: +2MB DMA, frees 32KB/part of SBUF).
    ae = big.tile([P, DT // 2, SQ], bf16, name="ae", tag="big")
    ao = big.tile([P, DT // 2, SQ], bf16, name="ao", tag="big")
    for sqc in range(NSQC):
        ss = slice(sqc * SQC, (sqc + 1) * SQC)
        for eo in range(DT):
            xve = strm.tile([P, SKT // 2 * P], bf16, name="xve", tag="xv",
                            bufs=4)
            xvo = strm.tile([P, SKT // 2 * P], bf16, name="xvo", tag="xv",
                            bufs=4)
            nc.sync.dma_start(xve[:], io["xv0"][:, eo, :])
            nc.sync.dma_start(xvo[:], io["xv1"][:, eo, :])
            pa = ps.tile([P, SQC], f32, tag="mm", name="pa", bufs=2)
            for sko in range(SKT):
                xvt = xve if sko % 2 == 0 else xvo
                aT = aT0 if sko % 2 == 0 else aT1
                skh = (sko // 2) * P
                nc.tensor.matmul(pa[:], xvt[:, skh:skh + P],
                                 aT[:, sko // 2, ss],
                                 start=(sko == 0), stop=(sko == SKT - 1))
            at = ae if eo % 2 == 0 else ao
            adst = at[:, eo // 2, ss]
            if eo % 2 == 0:
                nc.vector.tensor_copy(adst, pa[:])
            else:
                nc.scalar.activation(adst, pa[:], AF.Copy)
    if upto == 2:
        return

    # ---------------- phase 3: output projection ----------------
    # N[colchunk][parity of eo]: bf16, moving operand alternates tiles
    for dc in range(DC):
        ds_ = slice(dc * NB, (dc + 1) * NB)
        wn_t = []
        for g in range(2):
            wne = strm.tile([P, DT // 4, NB], bf16, name=f"wne{g}",
                            tag="wn", bufs=4)
            wno = strm.tile([P, DT // 4, NB], bf16, name=f"wno{g}",
                            tag="wn", bufs=4)
            e0 = g * (DT // 2)
            nc.sync.dma_start(wne[:], io["wn"][:, e0:e0 + DT // 2:2, ds_])
            nc.sync.dma_start(wno[:], io["wn"][:, e0 + 1:e0 + DT // 2:2, ds_])
            wn_t.append((wne, wno))
        for sqt in range(SQT):
            sq0 = sqt * P
            po = ps.tile([P, NB], f32, tag="mm", name="po", bufs=2)
            for eo in range(DT):
                at = ae if eo % 2 == 0 else ao
                wnt = wn_t[eo // (DT // 2)][eo % 2]
                nc.tensor.matmul(
                    po[:], at[:, eo // 2, sq0:sq0 + P],
                    wnt[:, (eo % (DT // 2)) // 2, :],
                    start=(eo == 0), stop=(eo == DT - 1))
            ot = strm.tile([P, NB], bf16, name="ot", tag="vo", bufs=2)
            nc.vector.tensor_tensor(ot[:], po[:], bp_t[:, ds_], ALU.add)
            nc.sync.dma_start(io["out"][sq0:sq0 + P, ds_], ot[:])


# ======================= host side =======================

def _to_pdt(x, inner=P):
    """[K, N] with K = KT*P -> [P, KT, N] (partition-major tiling)."""
    K, N = x.shape
    return np.ascontiguousarray(
        x.reshape(K // inner, inner, N).transpose(1, 0, 2))


def build_program(S=S, D=D, DEP=DEP, SQ=SQ, num_devices=NCORES,
                  repeats=1, pair=False, upto=3):
    from contextlib import ExitStack
    nc = bacc.Bacc("TRN2", target_bir_lowering=False, debug=False,
                   num_devices=num_devices)
    io = {}
    for name, (shape, dtp) in input_specs(S, D, DEP, SQ).items():
        io[name] = nc.dram_tensor(name, shape, dtp, kind="ExternalInput").ap()
    io["out"] = nc.dram_tensor("out", [SQ, DEP], bf16,
                               kind="ExternalOutput").ap()
    with tile.TileContext(nc) as tc:
        for _ in range(repeats):
            with ExitStack() as ctx:
                emit_attention(ctx, tc, io, S, D, DEP, SQ, upto=upto)
    nc.compile()
    return nc


def make_in_maps(query, keys, values, Wq, bq, Wk, bk, Wv, bv, Wd, bd):
    """Per-core input maps (numpy f32) from the full-problem arrays."""
    f8 = np.float64
    # colbias[b, sk] = keys[b] @ (Wk @ bq), exact in f64
    wkbq = (Wk.astype(f8) @ bq.astype(f8)).astype(np.float32)
    colbias = keys @ wkbq  # [B, S]
    # fold matrices (exact to f32)
    Mt = (Wk.astype(f8) @ Wq.astype(f8).T).astype(np.float32)  # M^T
    N = (Wv.astype(f8) @ Wd.astype(f8)).astype(np.float32)
    bp = (bv.astype(f8) @ Wd.astype(f8) + bd.astype(f8)).astype(np.float32)

    shared = {
        "wm": _to_pdt(Mt),
        "wn": _to_pdt(N).astype(ml_dtypes.bfloat16),
        "bp": np.ascontiguousarray(np.broadcast_to(bp, (P, DEP))),
    }

    SKT, DT = S // P, D // P
    batch_part = []
    for b in range(B):
        # xv_par[p, eo, skoh*P + j] = values[(2*skoh + par)*P + p, eo*P + j]
        v4 = values[b].reshape(SKT, P, DT, P)
        xv0 = np.ascontiguousarray(
            v4[0::2].transpose(1, 2, 0, 3).reshape(P, DT, SKT // 2 * P)
        ).astype(ml_dtypes.bfloat16)
        xv1 = np.ascontiguousarray(
            v4[1::2].transpose(1, 2, 0, 3).reshape(P, DT, SKT // 2 * P)
        ).astype(ml_dtypes.bfloat16)
        batch_part.append({
            "xk": _to_pdt(np.ascontiguousarray(keys[b].T)),
            "xv0": xv0,
            "xv1": xv1,
            "colbias": np.ascontiguousarray(
                np.broadcast_to(colbias[b], (P, S))).astype(
                    ml_dtypes.bfloat16),
        })

    in_maps = []
    for c in range(NCORES):
        b, qh = divmod(c, 2)
        qT = np.ascontiguousarray(query[b, qh * SQ:(qh + 1) * SQ].T)
        m = {"xq": _to_pdt(qT)}
        m.update(batch_part[b])
        m.update(shared)
        in_maps.append(m)
    return in_maps


_CACHE = {}


def kernel(query, keys, values, Wq, bq, Wk, bk, Wv, bv, Wd, bd):
    args = [np.asarray(a, np.float32) for a in
            (query, keys, values, Wq, bq, Wk, bk, Wv, bv, Wd, bd)]

    if "nc" not in _CACHE:
        _CACHE["nc"] = build_program()
    nc = _CACHE["nc"]

    in_maps = make_in_maps(*args)
    outs = _run_spmd(nc, in_maps)

    out = np.empty((B, S, DEP), np.float32)
    for c in range(NCORES):
        b, qh = divmod(c, 2)
        out[b, qh * SQ:(qh + 1) * SQ] = outs[c].astype(np.float32)
    return out


def _get_runner(nc):
    """Build (once) a cached jitted shard_map executor for nc."""
    if "runner" in _CACHE:
        return _CACHE["runner"]
    import jax
    import concourse.mybir as mybir_
    from concourse import bass2jax
    from concourse.bass2jax import _bass_exec_p, install_neuronx_cc_hook
    from jax.experimental.shard_map import shard_map
    from jax.sharding import Mesh, PartitionSpec

    install_neuronx_cc_hook()
    in_names, out_names, out_avals, zero_outs = [], [], [], []
    for alloc in nc.m.functions[0].allocations:
        if not isinstance(alloc, mybir_.MemoryLocationSet):
            continue
        name = alloc.memorylocations[0].name
        if alloc.kind == "ExternalInput":
            if nc.partition_id_tensor is None or \
                    name != nc.partition_id_tensor.name:
                in_names.append(name)
        elif alloc.kind == "ExternalOutput":
            out_names.append(name)
            shape = tuple(alloc.tensor_shape)
            dtp = mybir_.dt.np(alloc.dtype)
            out_avals.append(jax.core.ShapedArray(shape, dtp))
            zero_outs.append(np.zeros(shape, dtp))
    n_params = len(in_names)
    n_outs = len(out_avals)
    all_names = in_names + out_names
    pname = nc.partition_id_tensor.name if nc.partition_id_tensor else None
    if pname is not None:
        all_names = all_names + [pname]
    donate = tuple(range(n_params, n_params + n_outs))

    def _body(*args):
        operands = list(args)
        if pname is not None:
            operands.append(bass2jax.partition_id_tensor())
        outs = _bass_exec_p.bind(
            *operands,
            out_avals=tuple(out_avals),
            in_names=tuple(all_names),
            out_names=tuple(out_names),
            lowering_input_output_aliases=(),
            sim_require_finite=True,
            sim_require_nnan=True,
            nc=nc,
        )
        return tuple(outs)

    devices = jax.devices()[:NCORES]
    mesh = Mesh(np.asarray(devices), ("core",))
    in_specs = (PartitionSpec("core"),) * (n_params + n_outs)
    out_specs = (PartitionSpec("core"),) * n_outs
    sharded = jax.jit(
        shard_map(_body, mesh=mesh, in_specs=in_specs, out_specs=out_specs,
                  check_rep=False),
        donate_argnums=donate, keep_unused=True)
    runner = (sharded, in_names, out_names, zero_outs)
    _CACHE["runner"] = runner
    return runner


def _run_spmd(nc, in_maps):
    """Run nc on NCORES devices; returns list of per-core 'out' arrays."""
    sharded, in_names, out_names, zero_outs = _get_runner(nc)
    concat_in = [
        np.concatenate([np.asarray(m[name]) for m in in_maps], axis=0)
        for name in in_names
    ]
    concat_zeros = [
        np.zeros((NCORES * z.shape[0], *z.shape[1:]), z.dtype)
        for z in zero_outs
    ]
    out_arrs = sharded(*concat_in, *concat_zeros)
    oi = out_names.index("out")
    full = np.asarray(out_arrs[oi])
    per = full.reshape(NCORES, full.shape[0] // NCORES, *full.shape[1:])
    return [per[c] for c in range(NCORES)]


# revision 48
# speedup vs baseline: 1.1549x; 1.1549x over previous
"""Trainium2 Bass kernel for nn_Attention_47493748359201.

Single-head attention: q/k/v projections -> softmax(q k^T) v -> output proj.
Full shapes: query/keys/values [4, 2048, 1024], weights [1024, 1024].

Sharding: 8 cores = (batch, query-half). Each core handles its own
1024-row query slice against the full 2048 keys of its batch; no
collectives.

Algebraic folds (exact, by matmul associativity):
  - scores = (Xq Wq)(Xk Wk)^T = Xq M Xk^T with M = Wq Wk^T precomputed
    on the host.  The kernel projects keys once through M^T (same cost
    as the old K projection) and uses raw Xq^T as the score stationary.
    The Q projection disappears.
  - out = attn (Xv Wv + bv) Wd + bd = attn Xv N + b' with N = Wv Wd and
    b' = bv Wd + bd precomputed on the host.  The attend matmul uses raw
    Xv (bf16) as its stationary.  The V projection disappears.
  Old tensor-engine work per core: kproj+qproj+vproj+scores+attend+out
  = 273us ideal; new: t''proj+scores+attend+out = 191us ideal.

Precision: single-pass fp32r (11-bit mantissa) for the t'' projection
and the score matmul; bf16 for attend and output-proj (softmax weights
in [0,1]; Xv and attended tolerate 8-bit mantissa).

Bias handling (exact):
  - bk: the scores term q.bk is constant along the key axis -> drops out
    of softmax; bq.bk is a global constant -> drops too.
  - bq: enters only through colbias[sk] = bq . k[sk] = keys[sk].(Wk bq),
    computed exactly on the host (tiny matvec) and added to scores.
  - bv, bd: fold into b' = bv Wd + bd, added (f32) during the output
    PSUM eviction.

Layout: everything SBUF-resident (no DRAM staging).  One "big" pool
with eight 16KB/partition slots rotates the long-lived tensors; ae/ao
(attended^T) reuse t'' slots after the score phase.

HW-measured scheduling rules this kernel is built around (verified by
same-window A/B on the device; the cost model does NOT predict them):
  - Split every long-lived tensor into even/odd tiles along the axis its
    consumer's matmul chain cycles through: alternating the moving
    operand between two SBUF tiles streams ~2x faster.
  - Sequential psum accumulation chains only.  Interleaving two chains
    or widening chunks to 512 both measured ~25% slower.
  - Evictions must alternate engines (ACT/DVE): a single-engine eviction
    stream gates psum recycling.
  - Keep DMAs on the sync queue in fine per-do slices; the gpsimd queue
    and consolidated strided DMAs both regressed.
"""
import sys

sys.path.insert(0, "/opt/trn_rl_repo")

import numpy as np
import ml_dtypes

import concourse.bass as bass
import concourse.mybir as mybir
import concourse.tile as tile
from concourse import bacc
from concourse.masks import make_identity

P = 128
NB = 512  # matmul moving free dim (one PSUM bank of f32)
AF = mybir.ActivationFunctionType
ALU = mybir.AluOpType
dt = mybir.dt
f32 = dt.float32
f32r = dt.float32r
bf16 = dt.bfloat16

# full-problem constants
B, S, D, H, DEP = 4, 2048, 1024, 1024, 1024
NCORES = 8
SQ = B * S // NCORES  # 1024 query rows per core


def input_specs(S=S, D=D, DEP=DEP, SQ=SQ):
    """name -> (shape, mybir dtype) for the per-core DRAM inputs."""
    DT, SKT = D // P, S // P
    return {
        "xq": ([P, DT, SQ], f32r),       # query^T  (d-major tiles)
        "xk": ([P, DT, S], f32r),        # keys^T   (d-major tiles)
        # values, regrouped per e-block and split by sk-tile parity so the
        # attend phase streams one contiguous [P, SKT/2*P] slice per eo:
        # xv0[p, eo, skoh*P + j] = values[(2*skoh)*P + p, eo*P + j]
        "xv0": ([P, DT, SKT // 2 * P], bf16),
        "xv1": ([P, DT, SKT // 2 * P], bf16),
        "wm": ([P, DT, D], f32r),        # M^T = Wk Wq^T (d_in-major)
        "wn": ([P, DT, DEP], bf16),      # N = Wv Wd     (d-major)
        "bp": ([P, DEP], f32),           # b' = bv Wd + bd (broadcast)
        "colbias": ([P, S], bf16),
    }


def emit_attention(ctx, tc, io, S=S, D=D, DEP=DEP, SQ=SQ, upto=3):
    """Emit the per-core attention program. io: dict name -> bass.AP
    (input_specs() names plus "out" [SQ, DEP] f32).
    upto: emit phases 0..upto only (for phase timing)."""
    nc = tc.nc
    DT, SKT, SQT = D // P, S // P, SQ // P
    SKC = S // NB          # score/key column chunks (4)
    S2 = S // 2            # t'' cols per half tile
    DC = DEP // NB         # output dep chunks
    NBQ = 512              # t''-projection column chunk
    SQC = min(NB, SQ)      # attend rhs chunk
    NSQC = SQ // SQC
    D2W = D // 2           # M-half width (512)
    NWH = D // D2W
    DT_W = D2W // P        # d tiles per M half
    XSB = 3                # xs-tag stream depth (chunks are 2MB at NBQ=512)

    # ---------------- resident SBUF (whole kernel) ----------------
    res = ctx.enter_context(tc.tile_pool(name="res", bufs=1))
    ident = res.tile([P, P], bf16)
    colbias = res.tile([P, S], bf16)
    nc.sync.dma_start(colbias[:], io["colbias"])
    bp_t = res.tile([P, DEP], f32)
    nc.sync.dma_start(bp_t[:], io["bp"])

    # big rotating slots (bufs=6 x 16KB/part).  Long-lived tensors are
    # split into even/odd tiles along the axis their consumer's matmul
    # chain cycles through, so consecutive matmuls alternate SBUF tiles.
    big = ctx.enter_context(tc.tile_pool(name="big", bufs=6))
    # t''[parity of do][colhalf]: scores chunk c cycles do, alt ke/ko
    ke0 = big.tile([P, DT // 2, S2], f32r, tag="big")
    ko0 = big.tile([P, DT // 2, S2], f32r, tag="big")
    ke1 = big.tile([P, DT // 2, S2], f32r, tag="big")
    ko1 = big.tile([P, DT // 2, S2], f32r, tag="big")
    # attnT[parity of sko]: attend chain cycles sko
    aT0 = big.tile([P, SKT // 2, SQ], bf16, tag="big")
    aT1 = big.tile([P, SKT // 2, SQ], bf16, tag="big")

    ps = ctx.enter_context(tc.tile_pool(name="ps", bufs=1, space="PSUM"))
    # stream pool: persistent tags -> DMA prefetch crosses phase boundaries
    strm = ctx.enter_context(tc.tile_pool(name="strm", bufs=1))

    ident_f = strm.tile([P, P], f32, name="ident_f", tag="vo", bufs=2)
    make_identity(nc, ident_f[:])
    nc.vector.tensor_copy(ident[:], ident_f[:])

    def load_whalf(nm, w_ap, half, dtp=f32r):
        """One M half as an even/odd-do pair of tiles."""
        te = strm.tile([P, DT // 2, D2W], dtp, name=nm + "e", tag="w",
                       bufs=4)
        to = strm.tile([P, DT // 2, D2W], dtp, name=nm + "o", tag="w",
                       bufs=4)
        hs = slice(half * D2W, (half + 1) * D2W)
        for do in range(DT):
            t = te if do % 2 == 0 else to
            nc.sync.dma_start(t[:, do // 2, :], w_ap[:, do, hs])
        return te, to

    def load_x_chunk(x_ap, c, ncols=NBQ):
        cs = slice(c * ncols, (c + 1) * ncols)
        xe = strm.tile([P, DT // 2, ncols], f32r, name="xe", tag="xs",
                       bufs=XSB)
        xo = strm.tile([P, DT // 2, ncols], f32r, name="xo", tag="xs",
                       bufs=XSB)
        for do in range(DT):
            t = xe if do % 2 == 0 else xo
            nc.sync.dma_start(t[:, do // 2, :], x_ap[:, do, cs])
        return xe, xo

    def proj(w_halves, x_ap, dst_of, ncols, first_x=None):
        """dst[i, col] = W^T @ x, single-pass f32r.
        dst_of(c) -> (tile, col_slice) eviction target for chunk c;
        dst tile is an (even, odd) i-parity pair."""
        for c in range(ncols // NBQ):
            xe, xo = first_x if (c == 0 and first_x is not None) \
                else load_x_chunk(x_ap, c)
            for ho in range(DT):
                pt = ps.tile([P, NBQ], f32, tag="ps8", name="pt", bufs=8)
                we, wo = w_halves[ho // DT_W]
                hs = slice((ho % DT_W) * P, (ho % DT_W + 1) * P)
                for do in range(DT):
                    wt = we if do % 2 == 0 else wo
                    xt = xe if do % 2 == 0 else xo
                    nc.tensor.matmul(pt[:], wt[:, do // 2, hs],
                                     xt[:, do // 2, :],
                                     start=(do == 0), stop=(do == DT - 1))
                dst_pair, cs = dst_of(c)
                dst = dst_pair[ho % 2][:, ho // 2, cs]
                # alternate eviction engine: halves the serial drain at
                # phase handoffs (consumers wait on the dst tile writers)
                if ho % 2 == 0:
                    nc.scalar.activation(dst, pt[:], AF.Copy)
                else:
                    nc.vector.tensor_copy(dst, pt[:])

    # ------------- phase 0: t'' = M^T-projection of keys -------------
    # first M half, then the first xk chunk, then the second half, so
    # the first psum group starts after ~3MB of DMA instead of ~5MB
    wm_h = [load_whalf("wm0", io["wm"], 0)]
    kfirst = load_x_chunk(io["xk"], 0)
    wm_h += [load_whalf(f"wm{h}", io["wm"], h) for h in range(1, NWH)]

    def k_dst(c):
        pair = (ke0, ko0) if c < S2 // NBQ else (ke1, ko1)
        c0 = c % (S2 // NBQ)
        return pair, slice(c0 * NBQ, (c0 + 1) * NBQ)

    proj(wm_h, io["xk"], k_dst, S, first_x=kfirst)
    if upto == 0:
        return

    # ---------------- phase 1: scores + softmax + transpose ----------------
    # Software-pipelined: sqt's transposes are emitted AFTER sqt+1's score
    # chains so the in-order tensor queue never waits on sqt's softmax.
    # Engine split per chunk: PE chain -> Pool (colbias add + max) -> ACT
    # (exp).  DVE only runs the tiny normalize chain + half the rescales
    # and aT copies; without this the phase ran serial at ~123us/body
    # (PE 61 + DVE 60 + ACT 15 all on the critical path).
    EW = min(S, 2 * NB)  # columns per e tile
    with tc.tile_pool(name="soft", bufs=2) as soft:
        def score_block(sqt):
            sq0 = sqt * P
            # stationary for this query block: xq[:, do, sq0:sq0+P],
            # split by do parity
            xqe = strm.tile([P, DT // 2, P], f32r, name="xqe", tag="xqe",
                            bufs=2)
            xqo = strm.tile([P, DT // 2, P], f32r, name="xqo", tag="xqo",
                            bufs=2)
            # one strided DMA per parity: tiny per-do slices are DMA-
            # latency-bound (~2.3us each regardless of size)
            nc.sync.dma_start(xqe[:], io["xq"][:, 0::2, sq0:sq0 + P])
            nc.sync.dma_start(xqo[:], io["xq"][:, 1::2, sq0:sq0 + P])
            es_ = [strm.tile([P, EW], bf16, name="e", tag="e", bufs=4)
                   for _ in range(S // EW)]
            nm_arr = soft.tile([P, SKC], f32, name="nm_arr")
            es_arr = soft.tile([P, SKC], f32, name="es_arr")
            for c in range(SKC):
                cs = slice(c * NB, (c + 1) * NB)
                kpair = (ke0, ko0) if c < SKC // 2 else (ke1, ko1)
                kcs = slice((c % (SKC // 2)) * NB, (c % (SKC // 2) + 1) * NB)
                sch = ps.tile([P, NB], f32, tag="ps8", name="sch", bufs=8)
                for do in range(DT):
                    xqt = xqe if do % 2 == 0 else xqo
                    nc.tensor.matmul(sch[:], xqt[:, do // 2, :],
                                     kpair[do % 2][:, do // 2, kcs],
                                     start=(do == 0), stop=(do == DT - 1))
                # colbias add evicts PSUM->SBUF in the same pass: the
                # max and exp then read SBUF (PSUM-side reads cost ~2x on
                # DVE/ACT) and the PSUM bank frees one op earlier
                sb = strm.tile([P, NB], f32, name="sb", tag="sb", bufs=4)
                nc.vector.tensor_tensor(sb[:], sch[:], colbias[:, cs],
                                        ALU.add)
                nc.vector.reduce_max(out=nm_arr[:, c:c + 1], in_=sb[:],
                                     axis=mybir.AxisListType.X, negate=True)
                ei = es_[(c * NB) // EW]
                ecs = slice((c * NB) % EW, (c * NB) % EW + NB)
                nc.scalar.activation(ei[:, ecs], sb[:], AF.Exp,
                                     bias=nm_arr[:, c:c + 1],
                                     accum_out=es_arr[:, c:c + 1])
            # global max and per-quarter rescale factors
            nmax = soft.tile([P, 1], f32, name="nmax")
            nc.vector.tensor_reduce(out=nmax[:], in_=nm_arr[:],
                                    op=ALU.min, axis=mybir.AxisListType.X)
            dm = soft.tile([P, SKC], f32, name="dm")
            nc.vector.tensor_scalar_sub(dm[:], nm_arr[:], nmax[:])
            fq = soft.tile([P, SKC], f32, name="fq")
            nc.scalar.activation(fq[:], dm[:], AF.Exp, scale=-1.0)
            wsum = soft.tile([P, SKC], f32, name="wsum")
            nc.vector.tensor_tensor(wsum[:], fq[:], es_arr[:], ALU.mult)
            esum = soft.tile([P, 1], f32, name="esum")
            nc.vector.reduce_sum(out=esum[:], in_=wsum[:],
                                 axis=mybir.AxisListType.X)
            recip = soft.tile([P, 1], f32, name="recip")
            nc.vector.reciprocal(recip[:], esum[:])
            r_arr = soft.tile([P, SKC], f32, name="r_arr")
            nc.vector.tensor_scalar_mul(r_arr[:], fq[:], recip[:])
            for c in range(SKC):
                ei = es_[(c * NB) // EW]
                ecs = slice((c * NB) % EW, (c * NB) % EW + NB)
                nc.vector.tensor_scalar_mul(ei[:, ecs], ei[:, ecs],
                                            r_arr[:, c:c + 1])
            return es_, sq0

        def transpose_block(es_, sq0):
            # two same-parity transposes share one [P, 2P] psum tile and
            # drain in ONE strided 256-col copy (8 copies/block, not 16)
            for par in range(2):
                aT = aT0 if par == 0 else aT1
                for k2 in range(SKT // 4):
                    ptr = ps.tile([P, 2, P], bf16, tag="ps8", name="ptr",
                                  bufs=8)
                    for h in range(2):
                        sko = (2 * k2 + h) * 2 + par
                        ei = es_[(sko * P) // EW]
                        ecs = slice((sko * P) % EW, (sko * P) % EW + P)
                        nc.tensor.transpose(ptr[:, h, :], ei[:, ecs],
                                            ident[:])
                    dst = aT[:, 2 * k2:2 * k2 + 2, sq0:sq0 + P]
                    if (par + k2) % 2 == 0:
                        nc.vector.tensor_copy(dst, ptr[:])
                    else:
                        nc.scalar.activation(dst, ptr[:], AF.Copy)

        pend = None
        for sqt in range(SQT):
            blk = score_block(sqt)
            if pend is not None:
                transpose_block(*pend)
            pend = blk
        transpose_block(*pend)
    if upto == 1:
        return

    # ------------- phase 2: attend (attG^T into t'' slots) -------------
    # attG^T[parity of eo]: outproj's stationary cycles eo.  The xv
    # stationary streams from DRAM one eo-block at a time (re-loaded per
    # sqc pass: +2MB DMA, frees 32KB/part of SBUF).
    ae = big.tile([P, DT // 2, SQ], bf16, name="ae", tag="big")
    ao = big.tile([P, DT // 2, SQ], bf16, name="ao", tag="big")
    for sqc in range(NSQC):
        ss = slice(sqc * SQC, (sqc + 1) * SQC)
        for eo in range(DT):
            xve = strm.tile([P, SKT // 2 * P], bf16, name="xve", tag="xv",
                            bufs=4)
            xvo = strm.tile([P, SKT // 2 * P], bf16, name="xvo", tag="xv",
                            bufs=4)
            nc.sync.dma_start(xve[:], io["xv0"][:, eo, :])
            nc.sync.dma_start(xvo[:], io["xv1"][:, eo, :])
            pa = ps.tile([P, SQC], f32, tag="ps8", name="pa", bufs=8)
            for sko in range(SKT):
                xvt = xve if sko % 2 == 0 else xvo
                aT = aT0 if sko % 2 == 0 else aT1
                skh = (sko // 2) * P
                nc.tensor.matmul(pa[:], xvt[:, skh:skh + P],
                                 aT[:, sko // 2, ss],
                                 start=(sko == 0), stop=(sko == SKT - 1))
            at = ae if eo % 2 == 0 else ao
            adst = at[:, eo // 2, ss]
            if eo % 2 == 0:
                nc.vector.tensor_copy(adst, pa[:])
            else:
                nc.scalar.activation(adst, pa[:], AF.Copy)
    if upto == 2:
        return

    # ---------------- phase 3: output projection ----------------
    # N[colchunk][parity of eo]: bf16, moving operand alternates tiles
    for dc in range(DC):
        ds_ = slice(dc * NB, (dc + 1) * NB)
        wn_t = []
        for g in range(2):
            wne = strm.tile([P, DT // 4, NB], bf16, name=f"wne{g}",
                            tag="wn", bufs=4)
            wno = strm.tile([P, DT // 4, NB], bf16, name=f"wno{g}",
                            tag="wn", bufs=4)
            e0 = g * (DT // 2)
            nc.sync.dma_start(wne[:], io["wn"][:, e0:e0 + DT // 2:2, ds_])
            nc.sync.dma_start(wno[:], io["wn"][:, e0 + 1:e0 + DT // 2:2, ds_])
            wn_t.append((wne, wno))
        for sqt in range(SQT):
            sq0 = sqt * P
            po = ps.tile([P, NB], f32, tag="ps8", name="po", bufs=8)
            for eo in range(DT):
                at = ae if eo % 2 == 0 else ao
                wnt = wn_t[eo // (DT // 2)][eo % 2]
                nc.tensor.matmul(
                    po[:], at[:, eo // 2, sq0:sq0 + P],
                    wnt[:, (eo % (DT // 2)) // 2, :],
                    start=(eo == 0), stop=(eo == DT - 1))
            ot = strm.tile([P, NB], bf16, name="ot", tag="vo", bufs=2)
            nc.vector.tensor_tensor(ot[:], po[:], bp_t[:, ds_], ALU.add)
            nc.sync.dma_start(io["out"][sq0:sq0 + P, ds_], ot[:])


# ======================= host side =======================

def _to_pdt(x, inner=P):
    """[K, N] with K = KT*P -> [P, KT, N] (partition-major tiling)."""
    K, N = x.shape
    return np.ascontiguousarray(
        x.reshape(K // inner, inner, N).transpose(1, 0, 2))


def build_program(S=S, D=D, DEP=DEP, SQ=SQ, num_devices=NCORES,
                  repeats=1, pair=False, upto=3):
    from contextlib import ExitStack
    nc = bacc.Bacc("TRN2", target_bir_lowering=False, debug=False,
                   num_devices=num_devices)
    io = {}
    for name, (shape, dtp) in input_specs(S, D, DEP, SQ).items():
        io[name] = nc.dram_tensor(name, shape, dtp, kind="ExternalInput").ap()
    io["out"] = nc.dram_tensor("out", [SQ, DEP], bf16,
                               kind="ExternalOutput").ap()
    with tile.TileContext(nc) as tc:
        for _ in range(repeats):
            with ExitStack() as ctx:
                emit_attention(ctx, tc, io, S, D, DEP, SQ, upto=upto)
    nc.compile()
    return nc


def make_in_maps(query, keys, values, Wq, bq, Wk, bk, Wv, bv, Wd, bd):
    """Per-core input maps (numpy f32) from the full-problem arrays."""
    f8 = np.float64
    # colbias[b, sk] = keys[b] @ (Wk @ bq), exact in f64
    wkbq = (Wk.astype(f8) @ bq.astype(f8)).astype(np.float32)
    colbias = keys @ wkbq  # [B, S]
    # fold matrices (exact to f32)
    Mt = (Wk.astype(f8) @ Wq.astype(f8).T).astype(np.float32)  # M^T
    N = (Wv.astype(f8) @ Wd.astype(f8)).astype(np.float32)
    bp = (bv.astype(f8) @ Wd.astype(f8) + bd.astype(f8)).astype(np.float32)

    shared = {
        "wm": _to_pdt(Mt),
        "wn": _to_pdt(N).astype(ml_dtypes.bfloat16),
        "bp": np.ascontiguousarray(np.broadcast_to(bp, (P, DEP))),
    }

    SKT, DT = S // P, D // P
    batch_part = []
    for b in range(B):
        # xv_par[p, eo, skoh*P + j] = values[(2*skoh + par)*P + p, eo*P + j]
        v4 = values[b].reshape(SKT, P, DT, P)
        xv0 = np.ascontiguousarray(
            v4[0::2].transpose(1, 2, 0, 3).reshape(P, DT, SKT // 2 * P)
        ).astype(ml_dtypes.bfloat16)
        xv1 = np.ascontiguousarray(
            v4[1::2].transpose(1, 2, 0, 3).reshape(P, DT, SKT // 2 * P)
        ).astype(ml_dtypes.bfloat16)
        batch_part.append({
            "xk": _to_pdt(np.ascontiguousarray(keys[b].T)),
            "xv0": xv0,
            "xv1": xv1,
            "colbias": np.ascontiguousarray(
                np.broadcast_to(colbias[b], (P, S))).astype(
                    ml_dtypes.bfloat16),
        })

    in_maps = []
    for c in range(NCORES):
        b, qh = divmod(c, 2)
        qT = np.ascontiguousarray(query[b, qh * SQ:(qh + 1) * SQ].T)
        m = {"xq": _to_pdt(qT)}
        m.update(batch_part[b])
        m.update(shared)
        in_maps.append(m)
    return in_maps


_CACHE = {}


def kernel(query, keys, values, Wq, bq, Wk, bk, Wv, bv, Wd, bd):
    args = [np.asarray(a, np.float32) for a in
            (query, keys, values, Wq, bq, Wk, bk, Wv, bv, Wd, bd)]

    if "nc" not in _CACHE:
        _CACHE["nc"] = build_program()
    nc = _CACHE["nc"]

    in_maps = make_in_maps(*args)
    outs = _run_spmd(nc, in_maps)

    out = np.empty((B, S, DEP), np.float32)
    for c in range(NCORES):
        b, qh = divmod(c, 2)
        out[b, qh * SQ:(qh + 1) * SQ] = outs[c].astype(np.float32)
    return out


def _get_runner(nc):
    """Build (once) a cached jitted shard_map executor for nc."""
    if "runner" in _CACHE:
        return _CACHE["runner"]
    import jax
    import concourse.mybir as mybir_
    from concourse import bass2jax
    from concourse.bass2jax import _bass_exec_p, install_neuronx_cc_hook
    from jax.experimental.shard_map import shard_map
    from jax.sharding import Mesh, PartitionSpec

    install_neuronx_cc_hook()
    in_names, out_names, out_avals, zero_outs = [], [], [], []
    for alloc in nc.m.functions[0].allocations:
        if not isinstance(alloc, mybir_.MemoryLocationSet):
            continue
        name = alloc.memorylocations[0].name
        if alloc.kind == "ExternalInput":
            if nc.partition_id_tensor is None or \
                    name != nc.partition_id_tensor.name:
                in_names.append(name)
        elif alloc.kind == "ExternalOutput":
            out_names.append(name)
            shape = tuple(alloc.tensor_shape)
            dtp = mybir_.dt.np(alloc.dtype)
            out_avals.append(jax.core.ShapedArray(shape, dtp))
            zero_outs.append(np.zeros(shape, dtp))
    n_params = len(in_names)
    n_outs = len(out_avals)
    all_names = in_names + out_names
    pname = nc.partition_id_tensor.name if nc.partition_id_tensor else None
    if pname is not None:
        all_names = all_names + [pname]
    donate = tuple(range(n_params, n_params + n_outs))

    def _body(*args):
        operands = list(args)
        if pname is not None:
            operands.append(bass2jax.partition_id_tensor())
        outs = _bass_exec_p.bind(
            *operands,
            out_avals=tuple(out_avals),
            in_names=tuple(all_names),
            out_names=tuple(out_names),
            lowering_input_output_aliases=(),
            sim_require_finite=True,
            sim_require_nnan=True,
            nc=nc,
        )
        return tuple(outs)

    devices = jax.devices()[:NCORES]
    mesh = Mesh(np.asarray(devices), ("core",))
    in_specs = (PartitionSpec("core"),) * (n_params + n_outs)
    out_specs = (PartitionSpec("core"),) * n_outs
    sharded = jax.jit(
        shard_map(_body, mesh=mesh, in_specs=in_specs, out_specs=out_specs,
                  check_rep=False),
        donate_argnums=donate, keep_unused=True)
    runner = (sharded, in_names, out_names, zero_outs)
    _CACHE["runner"] = runner
    return runner


def _run_spmd(nc, in_maps):
    """Run nc on NCORES devices; returns list of per-core 'out' arrays."""
    sharded, in_names, out_names, zero_outs = _get_runner(nc)
    concat_in = [
        np.concatenate([np.asarray(m[name]) for m in in_maps], axis=0)
        for name in in_names
    ]
    concat_zeros = [
        np.zeros((NCORES * z.shape[0], *z.shape[1:]), z.dtype)
        for z in zero_outs
    ]
    out_arrs = sharded(*concat_in, *concat_zeros)
    oi = out_names.index("out")
    full = np.asarray(out_arrs[oi])
    per = full.reshape(NCORES, full.shape[0] // NCORES, *full.shape[1:])
    return [per[c] for c in range(NCORES)]
